# revision 20
# baseline (speedup 1.0000x reference)
"""ANI-style per-species MLP (MoE hard routing) on 8 TRN2 NeuronCores.

Strategy:
  - Host: flatten atoms, sort by species, pad each species bucket to a
    multiple of 8*TILE, and deal equal per-species segments to each core.
    Every core therefore runs the SAME graph (SPMD) over its own atoms.
  - Device (per core): for each 512-atom tile of a species segment, run the
    4-layer MLP (384->160->128->96->1, CELU) with that species' weights only
    (4x less work than the dense reference). Matmuls in bf16 (fp32 PSUM
    accumulate), CELU via one ScalarE Exp pass + one fused custom DVE op:
        celu(v) = relu(v) + (min(exp(v), 1) - 1)       [v = z + b]
  - Host: scatter per-atom energies back, add the L4 bias, per-molecule sum.
"""

import os
from contextlib import ExitStack

import numpy as np
import ml_dtypes

import concourse.bacc as bacc
import concourse.mybir as mybir
import concourse.tile as tile
from concourse.bass_utils import run_bass_kernel_spmd

BF16 = ml_dtypes.bfloat16
F32 = np.float32

N_CORES = 8
TILE = 512
D_AEV = 384
DH1, DH2, DH3 = 160, 128, 96
N_SPECIES = 4

_ACT = mybir.ActivationFunctionType


# --------------------------------------------------------------------------- #
# Fused CELU custom DVE op: out = (min(in0, 1) - 1) + relu(in1 + s0)
#   in0 = exp(z + b) (SBUF f32, from ScalarE), in1 = z (PSUM f32), s0 = b.
# Result equals celu(z + b) exactly.
# --------------------------------------------------------------------------- #
def _register_celu_op():
    import concourse.dve_ops as dve_ops
    from concourse.dve_spec import Spec, Src0, Src1, C0, One, relu, minn, lower
    from concourse.dve_uop import DveOpSpec

    name = "CELU1_ANT"
    for op in dve_ops.OPS:
        if op.name == name:
            return op
    spec = Spec(
        body=(minn(Src0, One) - One) + relu(Src1 + C0),
        reference=lambda in0, in1, s0, s1, imm2: (np.minimum(in0, 1.0) - 1.0)
        + np.maximum(in1 + s0, 0.0),
    )
    row = dve_ops._CUSTOM_DVE_ROW_BASE + len(dve_ops.OPS)
    assert row < 0x20, "custom DVE row field overflow"
    shas = {}
    for ver in ("v3", "v4"):
        d = DveOpSpec(name=name, opcode=row, uops=lower(spec, ver=ver), rd1_en=True)
        shas[ver] = d.sha(ver)
    op = dve_ops.DveOp(name, spec, False, shas)
    dve_ops.OPS.append(op)
    dve_ops.CUSTOM_DVE_SPECS[name] = spec
    dve_ops._SUB_OPCODE_FOR_NAME[name] = row
    return op


# --------------------------------------------------------------------------- #
# Graph builder (one core's SPMD program).
# seg_tiles: number of 512-atom tiles per species segment. ncore = 512*sum.
# --------------------------------------------------------------------------- #
def build_graph(seg_tiles):
    celu_op = _register_celu_op()
    dt = mybir.dt
    ncore = TILE * int(sum(seg_tiles))

    nc = bacc.Bacc("TRN2", target_bir_lowering=False, debug=False)

    x_ext = nc.dram_tensor("x", [128, 3, ncore], dt.bfloat16, kind="ExternalInput")
    w1_ext = nc.dram_tensor("w1", [128, 12 * DH1], dt.bfloat16, kind="ExternalInput")
    w2a_ext = nc.dram_tensor("w2a", [128, 4 * DH2], dt.bfloat16, kind="ExternalInput")
    w2b_ext = nc.dram_tensor("w2b", [128, 4 * DH2], dt.bfloat16, kind="ExternalInput")
    w3_ext = nc.dram_tensor("w3", [128, 4 * DH3], dt.bfloat16, kind="ExternalInput")
    w4_ext = nc.dram_tensor("w4", [96, 4], dt.bfloat16, kind="ExternalInput")
    b1a_ext = nc.dram_tensor("b1a", [128, 4], dt.float32, kind="ExternalInput")
    b1b_ext = nc.dram_tensor("b1b", [128, 4], dt.float32, kind="ExternalInput")
    b2_ext = nc.dram_tensor("b2", [128, 4], dt.float32, kind="ExternalInput")
    b3_ext = nc.dram_tensor("b3", [96, 4], dt.float32, kind="ExternalInput")
    out_ext = nc.dram_tensor("out", [1, ncore], dt.float32, kind="ExternalOutput")

    with tile.TileContext(nc) as tc, ExitStack() as ctx:
        wpool = ctx.enter_context(tc.tile_pool(name="w", bufs=1))
        xpool = ctx.enter_context(tc.tile_pool(name="x", bufs=10))
        epool = ctx.enter_context(tc.tile_pool(name="e", bufs=4))
        spool = ctx.enter_context(tc.tile_pool(name="s", bufs=4))
        p1a = ctx.enter_context(tc.tile_pool(name="p1a", bufs=2, space="PSUM"))
        p1b = ctx.enter_context(tc.tile_pool(name="p1b", bufs=1, space="PSUM"))
        p2 = ctx.enter_context(tc.tile_pool(name="p2", bufs=2, space="PSUM"))
        p3 = ctx.enter_context(tc.tile_pool(name="p3", bufs=2, space="PSUM"))
        p4 = ctx.enter_context(tc.tile_pool(name="p4", bufs=1, space="PSUM"))

        # --- load weights/biases once ---
        w1_sb = wpool.tile([128, 12 * DH1], dt.bfloat16)
        w2a_sb = wpool.tile([128, 4 * DH2], dt.bfloat16)
        w2b_sb = wpool.tile([128, 4 * DH2], dt.bfloat16)
        w3_sb = wpool.tile([128, 4 * DH3], dt.bfloat16)
        w4_sb = wpool.tile([96, 4], dt.bfloat16)
        b1a_sb = wpool.tile([128, 4], dt.float32)
        b1b_sb = wpool.tile([128, 4], dt.float32)
        b2_sb = wpool.tile([128, 4], dt.float32)
        b3_sb = wpool.tile([96, 4], dt.float32)
        # w1 is the biggest blob and gates the first matmul — split it
        # across DMA queues so it lands ~4x sooner.
        for q in range(4):
            c0, c1 = q * 3 * DH1, (q + 1) * 3 * DH1
            nc.sync.dma_start(w1_sb[:, c0:c1], w1_ext[:, c0:c1])
        for sb, ext in [
            (w2a_sb, w2a_ext), (w2b_sb, w2b_ext),
            (w3_sb, w3_ext), (w4_sb, w4_ext), (b1a_sb, b1a_ext),
            (b1b_sb, b1b_ext), (b2_sb, b2_ext), (b3_sb, b3_ext),
        ]:
            nc.sync.dma_start(sb[:], ext[:])

        def celu(z_ap, bias, shape, tag):
            P = z_ap.shape[0]
            e = epool.tile(shape, dt.bfloat16, tag="e" + tag)
            nc.scalar.activation(e[0:P, :], z_ap, _ACT.Exp, bias=bias)
            sx = spool.tile(shape, dt.bfloat16, tag="s" + tag)
            nc.vector._custom_dve(
                celu_op, out=sx[0:P, :], in0=e[0:P, :], in1=z_ap, s0=bias
            )
            return sx

        def group_start(grp):
            for t in grp["tiles"]:
                xt = xpool.tile([128, 3, TILE], dt.bfloat16)
                # per-K-chunk DMAs parallelize across queues and let the
                # first matmul start as soon as chunk 0 lands
                for k in range(3):
                    nc.sync.dma_start(
                        xt[:, k, :], x_ext[:, k, t["g0"] : t["g0"] + TILE]
                    )
                t["xt"] = xt
            grp["z1b"] = p1b.tile([128, TILE], dt.float32, name="z1b", tag="z1b")

        def stage1(st):
            s = st["s"]
            z1a = p1a.tile([128, TILE], dt.float32)
            for k in range(3):
                base = (s * 3 + k) * DH1
                nc.tensor.matmul(
                    z1a[:], w1_sb[:, base : base + 128], st["xt"][:, k, :],
                    start=(k == 0), stop=(k == 2),
                )
            st["s1a"] = celu(z1a[:], b1a_sb[:, s : s + 1], [128, TILE], "1a")
            if st["j"] == 0:
                # L1 remainder (cols 128:160) for the whole group, packed
                # into one PSUM bank at partitions 32j (col-group concurrency)
                grp = st["grp"]
                gs = grp["gsize"]
                z1b = grp["z1b"]
                for j, t in enumerate(grp["tiles"]):
                    for k in range(3):
                        base = (s * 3 + k) * DH1 + 128
                        nc.tensor.matmul(
                            z1b[32 * j : 32 * j + 32, :],
                            w1_sb[:, base : base + 32],
                            t["xt"][:, k, :],
                            start=(k == 0), stop=(k == 2),
                            tile_position=(0, 32 * j),
                        )
                P = 32 * gs
                grp["s1b"] = celu(
                    z1b[0:P, :], b1b_sb[0:P, s : s + 1], [128, TILE], "1b"
                )

        def stage2(st):
            s, j = st["s"], st["j"]
            z2 = p2.tile([128, TILE], dt.float32)
            nc.tensor.matmul(
                z2[:], w2a_sb[:, s * DH2 : (s + 1) * DH2], st["s1a"][:],
                start=True, stop=False,
            )
            nc.tensor.matmul(
                z2[:],
                w2b_sb[32 * j : 32 * j + 32, s * DH2 : (s + 1) * DH2],
                st["grp"]["s1b"][32 * j : 32 * j + 32, :],
                start=False, stop=True,
                tile_position=(32 * j, 0),
            )
            st["s2"] = celu(z2[:], b2_sb[:, s : s + 1], [128, TILE], "2")

        def stage3(st):
            s = st["s"]
            z3 = p3.tile([96, TILE], dt.float32)
            nc.tensor.matmul(z3[:], w3_sb[:, s * DH3 : (s + 1) * DH3], st["s2"][:])
            st["s3"] = celu(z3[:], b3_sb[0:96, s : s + 1], [96, TILE], "3")

        def stage4(st):
            s, j, grp = st["s"], st["j"], st["grp"]
            if j == 0:
                grp["z4"] = p4.tile([128, TILE], dt.float32, name="z4", tag="z4")
            nc.tensor.matmul(
                grp["z4"][32 * j : 32 * j + 1, :],
                w4_sb[:, s : s + 1],
                st["s3"][:],
                tile_position=(0, 32 * j),
            )
            if j == grp["gsize"] - 1:
                gs = grp["gsize"]
                hi = 32 * (gs - 1) + 1
                en = spool.tile([128, TILE], dt.float32, tag="en")
                # compute engines require partition step 1: copy the whole
                # [0:hi] span (cost is free-dim driven), DMA the 4 live rows.
                nc.scalar.copy(en[0:hi, :], grp["z4"][0:hi, :])
                nc.sync.dma_start(
                    out_ext[0:1, grp["g0"] : grp["g0"] + gs * TILE].rearrange(
                        "p (a n) -> (p a) n", n=TILE
                    ),
                    en[0:hi:32, :],
                )

        tiles = []
        g0 = 0
        for s in range(N_SPECIES):
            nt = int(seg_tiles[s])
            for base in range(0, nt, 4):
                gs = min(4, nt - base)
                grp = {"s": s, "gsize": gs, "g0": g0, "tiles": []}
                for j in range(gs):
                    t = {"s": s, "g0": g0, "j": j, "grp": grp}
                    tiles.append(t)
                    grp["tiles"].append(t)
                    g0 += TILE

        # software-pipeline skew: tile t's layer l is emitted at step t + l,
        # so every engine always has independent work from adjacent tiles.
        n = len(tiles)
        for step in range(n + 3):
            if step < n:
                if tiles[step]["j"] == 0:
                    group_start(tiles[step]["grp"])
                stage1(tiles[step])
            if 0 <= step - 1 < n:
                stage2(tiles[step - 1])
            if 0 <= step - 2 < n:
                stage3(tiles[step - 2])
            if 0 <= step - 3 < n:
                stage4(tiles[step - 3])

    nc.compile()
    return nc


# --------------------------------------------------------------------------- #
# Host-side input prep / output unpack.
# --------------------------------------------------------------------------- #
def _prep_weights(W1, b1, W2, b2, W3, b3, W4, b4):
    # w1: [128, 12*DH1], column block (s*3+k) holds W1[s][128k:128k+128, :]
    w1 = np.empty((128, 12 * DH1), BF16)
    for s in range(4):
        for k in range(3):
            base = (s * 3 + k) * DH1
            w1[:, base : base + DH1] = W1[s, 128 * k : 128 * (k + 1), :].astype(BF16)
    w2a = np.empty((128, 4 * DH2), BF16)
    w2b = np.empty((128, 4 * DH2), BF16)  # rem weights replicated at 4 offsets
    w3 = np.empty((128, 4 * DH3), BF16)
    w4 = np.empty((96, 4), BF16)
    b1a = np.empty((128, 4), F32)
    b1b = np.empty((128, 4), F32)  # rem bias replicated at 4 offsets
    b2p = np.empty((128, 4), F32)
    b3p = np.empty((96, 4), F32)
    b4p = np.empty(4, F32)
    for s in range(4):
        w2a[:, s * DH2 : (s + 1) * DH2] = W2[s, :128, :].astype(BF16)
        w2b[:, s * DH2 : (s + 1) * DH2] = np.tile(W2[s, 128:, :], (4, 1)).astype(BF16)
        w3[:, s * DH3 : (s + 1) * DH3] = W3[s].astype(BF16)
        w4[:, s] = W4[s, :, 0].astype(BF16)
        b1a[:, s] = b1[s, :128]
        b1b[:, s] = np.tile(b1[s, 128:], 4)
        b2p[:, s] = b2[s]
        b3p[:, s] = b3[s]
        b4p[s] = b4[s, 0]
    return dict(w1=w1, w2a=w2a, w2b=w2b, w3=w3, w4=w4,
                b1a=b1a, b1b=b1b, b2=b2p, b3=b3p), b4p


def _route(species, aev):
    """Sort atoms by species, pad per species to 8*TILE multiples, deal to
    cores. Returns (x_per_core [8,128,3,ncore] bf16, slotmap [8,ncore] int64,
    seg_tiles [4])."""
    n = species.size
    sp = species.reshape(-1)
    x = aev.reshape(n, D_AEV)
    seg_tiles = []
    per_core_ids = []
    for s in range(N_SPECIES):
        ids = np.nonzero(sp == s)[0]
        t = max(1, int(np.ceil(len(ids) / (N_CORES * TILE))))
        seg_tiles.append(t)
        padded = np.full(N_CORES * t * TILE, -1, np.int64)
        padded[: len(ids)] = ids
        per_core_ids.append(padded.reshape(N_CORES, t * TILE))
    slotmap = np.concatenate(per_core_ids, axis=1)  # [8, ncore]
    ncore = slotmap.shape[1]

    x_bf = x.astype(BF16)
    x_cores = np.zeros((N_CORES, ncore, D_AEV), BF16)
    for i in range(N_CORES):
        valid = slotmap[i] >= 0
        x_cores[i, valid] = x_bf[slotmap[i][valid]]
    # device layout: [128, 3, ncore] with feature f = c*128 + p
    xT = np.ascontiguousarray(
        x_cores.reshape(N_CORES, ncore, 3, 128).transpose(0, 3, 2, 1)
    )
    return xT, slotmap, seg_tiles


_GRAPH_CACHE = {}


def kernel(species, aev, W1, b1, W2, b2, W3, b3, W4, b4):
    species = np.asarray(species)
    aev = np.asarray(aev, F32)
    B, A = species.shape

    xT, slotmap, seg_tiles = _route(species, aev)
    wmap, b4p = _prep_weights(
        np.asarray(W1, F32), np.asarray(b1, F32), np.asarray(W2, F32),
        np.asarray(b2, F32), np.asarray(W3, F32), np.asarray(b3, F32),
        np.asarray(W4, F32), np.asarray(b4, F32),
    )

    key = tuple(seg_tiles)
    if key not in _GRAPH_CACHE:
        _GRAPH_CACHE[key] = build_graph(seg_tiles)
    nc = _GRAPH_CACHE[key]

    in_maps = [{"x": xT[i], **wmap} for i in range(N_CORES)]
    res = run_bass_kernel_spmd(
        nc,
        in_maps,
        core_ids=list(range(N_CORES)),
        trace=bool(os.environ.get("ANI_TRACE")),
    )
    kernel.last_result = res
    if res.exec_time_ns is not None:
        print(f"HW exec time: {res.exec_time_ns} ns")

    n = B * A
    y_atoms = np.zeros(n, F32)
    for i in range(N_CORES):
        valid = slotmap[i] >= 0
        y_atoms[slotmap[i][valid]] = res.results[i]["out"][0][valid]
    y_atoms += b4p[species.reshape(-1)]
    return y_atoms.reshape(B, A).sum(axis=-1).astype(F32)
